# revision 5
# baseline (speedup 1.0000x reference)
"""NonNegLinear forward (eval path) on 8 Trainium2 NeuronCores.

reference:
    w = clip(weight, 0, 5)                       # [C, P]
    importance[b, p, c] = input[b, p] * w[c, p]  # [B, P, C]  (>= 0, threshold no-op)
    logits = importance.sum(axis=1) + bias       # [B, C]

Shapes: B=128, P=1024, C=1000, fp32.

Strategy: data-parallel over batch — each of the 8 cores handles 16 batch
rows and the full weight.  Per core the dominant cost is writing its
16*1024*1000*4 = 65.5 MB importance shard, so the kernel is laid out around
streaming those writes at HBM rate:

  - weight is loaded [c, p] (natural layout), transposed 128x128-blockwise on
    the TensorEngine into wT [p, c] tiles, with clip(0,5) fused into the
    PSUM->SBUF copy.
  - input shard is transposed the same way into inputT [p, b] columns.
  - importance tile [128p x 1000c] = wT_tile * input[b, p_tile] broadcast
    (per-partition scalar) on the Vector/Scalar engines, then one 4 MB DMA
    per batch row writes the [1024, 1000] block contiguously.
  - logits = inputT.T @ wT accumulated on the TensorEngine over the 8
    p-tiles, bias added as an extra K=1 matmul into the same PSUM tile.
"""

import os
import sys

import numpy as np

if "/opt/trn_rl_repo" not in sys.path:
    sys.path.insert(0, "/opt/trn_rl_repo")

import concourse.bass as bass
import concourse.mybir as mybir
import concourse.tile as tile
from concourse import bacc, masks
from concourse.bass_utils import run_bass_kernel_spmd

N_CORES = 8
B_FULL, P, C = 128, 1024, 1000
B = B_FULL // N_CORES  # 16 batch rows per core
PT = P // 128  # 8 p-tiles
F32 = mybir.dt.float32

LAST_RESULTS = None  # BassKernelResults of the most recent run (for test.py)


def _build_program():
    nc = bacc.Bacc()

    inp = nc.dram_tensor("input", [B, P], F32, kind="ExternalInput")
    weight = nc.dram_tensor("weight", [C, P], F32, kind="ExternalInput")
    bias_d = nc.dram_tensor("bias", [C], F32, kind="ExternalInput")
    imp = nc.dram_tensor("importance", [B, P, C], F32, kind="ExternalOutput")
    logits_d = nc.dram_tensor("logits", [B, C], F32, kind="ExternalOutput")

    with tile.TileContext(nc) as tc:
        with (
            tc.tile_pool(name="const", bufs=1) as const_pool,
            tc.tile_pool(name="wt", bufs=1) as wt_pool,
            tc.tile_pool(name="wstage", bufs=2) as wstage_pool,
            tc.tile_pool(name="out", bufs=3) as out_pool,
            tc.tile_pool(name="psum_t", bufs=4, space="PSUM") as psum_t_pool,
            tc.tile_pool(name="psum_l", bufs=2, space="PSUM") as psum_l_pool,
        ):
            identity = const_pool.tile([128, 128], F32, tag="identity")
            masks.make_identity(nc, identity[:])

            input_sb = const_pool.tile([B, P], F32, tag="input_sb")
            nc.sync.dma_start(input_sb[:], inp[:])

            bias_sb = const_pool.tile([1, C], F32, tag="bias_sb")
            nc.sync.dma_start(bias_sb[:], bias_d[None, :])

            ones = const_pool.tile([1, B], F32, tag="ones")
            nc.vector.memset(ones[:], 1.0)

            # inputT[p_i, pi*B + b] = input[b, pi*128 + p_i]
            inputT = const_pool.tile([128, PT * B], F32, tag="inputT")
            for pi in range(PT):
                pt = psum_t_pool.tile([128, 128], F32, tag="tp")
                nc.tensor.transpose(
                    pt[:, :B],
                    input_sb[:, pi * 128 : (pi + 1) * 128],
                    identity[:B, :B],
                )
                nc.vector.tensor_copy(inputT[:, pi * B : (pi + 1) * B], pt[:, :B])

            # wT tiles: wt[pi][p_i, c] = clip(weight[c, pi*128 + p_i], 0, 5)
            wt = [
                wt_pool.tile([128, C], F32, tag=f"wt{pi}", name=f"wt{pi}")
                for pi in range(PT)
            ]
            for ci in range((C + 127) // 128):
                cn = min(128, C - ci * 128)
                wstage = wstage_pool.tile([128, P], F32, tag="wstage")
                nc.sync.dma_start(
                    wstage[:cn, :], weight[ci * 128 : ci * 128 + cn, :]
                )
                for pi in range(PT):
                    pt = psum_t_pool.tile([128, 128], F32, tag="tp")
                    nc.tensor.transpose(
                        pt[:, :cn],
                        wstage[:cn, pi * 128 : (pi + 1) * 128],
                        identity[:cn, :cn],
                    )
                    # fused clip(x, 0, 5) on the PSUM->SBUF copy
                    nc.vector.tensor_scalar(
                        wt[pi][:, ci * 128 : ci * 128 + cn],
                        pt[:, :cn],
                        5.0,
                        0.0,
                        op0=mybir.AluOpType.min,
                        op1=mybir.AluOpType.max,
                    )

            # importance: one [128, PT*C] tile per batch row -> one 4 MB DMA
            for b in range(B):
                out_t = out_pool.tile([128, PT * C], F32, tag="out")
                for pi in range(PT):
                    dst = out_t[:, pi * C : (pi + 1) * C]
                    scal = inputT[:, pi * B + b : pi * B + b + 1]
                    if pi % 3 == 2:
                        nc.scalar.mul(dst, wt[pi][:], scal)
                    else:
                        nc.vector.tensor_scalar_mul(dst, wt[pi][:], scal)
                nc.sync.dma_start(
                    imp[b].rearrange("(pi p) c -> p pi c", p=128),
                    out_t[:].rearrange("p (pi c) -> p pi c", c=C),
                )

            # logits = inputT.T @ wT + bias, accumulated in PSUM
            logits_sb = const_pool.tile([B, C], F32, tag="logits_sb")
            for c0 in range(0, C, 512):
                cw = min(512, C - c0)
                pl = psum_l_pool.tile([B, 512], F32, tag="pl")
                for pi in range(PT):
                    nc.tensor.matmul(
                        pl[:, :cw],
                        lhsT=inputT[:, pi * B : (pi + 1) * B],
                        rhs=wt[pi][:, c0 : c0 + cw],
                        start=(pi == 0),
                        stop=False,
                    )
                nc.tensor.matmul(
                    pl[:, :cw],
                    lhsT=ones[:, :],
                    rhs=bias_sb[:, c0 : c0 + cw],
                    start=False,
                    stop=True,
                )
                nc.scalar.copy(logits_sb[:, c0 : c0 + cw], pl[:, :cw])
            nc.sync.dma_start(logits_d[:], logits_sb[:])

    nc.compile()
    return nc


_PROGRAM = None


def kernel(input, weight, bias):
    global _PROGRAM, LAST_RESULTS

    input = np.ascontiguousarray(input, dtype=np.float32)
    weight = np.ascontiguousarray(weight, dtype=np.float32)
    bias = np.ascontiguousarray(bias, dtype=np.float32)
    assert input.shape == (B_FULL, P) and weight.shape == (C, P)

    if _PROGRAM is None:
        _PROGRAM = _build_program()

    in_maps = [
        {
            "input": input[i * B : (i + 1) * B],
            "weight": weight,
            "bias": bias,
        }
        for i in range(N_CORES)
    ]
    res = run_bass_kernel_spmd(
        _PROGRAM,
        in_maps,
        core_ids=list(range(N_CORES)),
        trace=bool(int(os.environ.get("KERNEL_TRACE", "0"))),
    )
    LAST_RESULTS = res

    importance = np.concatenate([r["importance"] for r in res.results], axis=0)
    logits = np.concatenate([r["logits"] for r in res.results], axis=0)
    return importance, logits


# revision 7
# speedup vs baseline: 1.5202x; 1.5202x over previous
"""NonNegLinear forward (eval path) on 8 Trainium2 NeuronCores.

reference:
    w = clip(weight, 0, 5)                       # [C, P]
    importance[b, p, c] = input[b, p] * w[c, p]  # [B, P, C]  (>= 0, threshold no-op)
    logits = importance.sum(axis=1) + bias       # [B, C]

Shapes: B=128, P=1024, C=1000, fp32.

Strategy: data-parallel over batch — each of the 8 cores handles 16 batch
rows and the full weight.  Per core the dominant cost is writing its
16*1024*1000*4 = 65.5 MB importance shard, so the kernel is laid out around
streaming those writes at HBM rate:

  - weight is loaded [c, p] (natural layout), transposed 128x128-blockwise on
    the TensorEngine into wT [p, c] tiles, with clip(0,5) fused into the
    PSUM->SBUF copy.
  - input shard is transposed the same way into inputT [p, b] columns.
  - importance tile [128p x 1000c] = wT_tile * input[b, p_tile] broadcast
    (per-partition scalar) on the Vector/Scalar engines, then one 4 MB DMA
    per batch row writes the [1024, 1000] block contiguously.
  - logits = inputT.T @ wT accumulated on the TensorEngine over the 8
    p-tiles, bias added as an extra K=1 matmul into the same PSUM tile.
"""

import os
import sys

import numpy as np

if "/opt/trn_rl_repo" not in sys.path:
    sys.path.insert(0, "/opt/trn_rl_repo")

import concourse.bass as bass
import concourse.mybir as mybir
import concourse.tile as tile
from concourse import bacc, masks
from concourse.bass_utils import run_bass_kernel_spmd

N_CORES = 8
B_FULL, P, C = 128, 1024, 1000
B = B_FULL // N_CORES  # 16 batch rows per core
PT = P // 128  # 8 p-tiles
F32 = mybir.dt.float32

LAST_RESULTS = None  # BassKernelResults of the most recent run (for test.py)


def _build_program(b_rows=B):
    nc = bacc.Bacc()

    inp = nc.dram_tensor("input", [B, P], F32, kind="ExternalInput")
    weight = nc.dram_tensor("weight", [C, P], F32, kind="ExternalInput")
    bias_d = nc.dram_tensor("bias", [C], F32, kind="ExternalInput")
    imp = nc.dram_tensor("importance", [B, P, C], F32, kind="ExternalOutput")
    logits_d = nc.dram_tensor("logits", [B, C], F32, kind="ExternalOutput")

    with tile.TileContext(nc) as tc:
        with (
            tc.tile_pool(name="const", bufs=1) as const_pool,
            tc.tile_pool(name="wt", bufs=1) as wt_pool,
            tc.tile_pool(name="wstage", bufs=2) as wstage_pool,
            tc.tile_pool(name="out", bufs=3) as out_pool,
            tc.tile_pool(name="psum_t", bufs=4, space="PSUM") as psum_t_pool,
            tc.tile_pool(name="psum_l", bufs=2, space="PSUM") as psum_l_pool,
        ):
            identity = const_pool.tile([128, 128], F32, tag="identity")
            masks.make_identity(nc, identity[:])

            input_sb = const_pool.tile([B, P], F32, tag="input_sb")
            nc.sync.dma_start(input_sb[:], inp[:])

            bias_sb = const_pool.tile([1, C], F32, tag="bias_sb")
            nc.sync.dma_start(bias_sb[:], bias_d[None, :])

            ones = const_pool.tile([1, B], F32, tag="ones")
            nc.vector.memset(ones[:], 1.0)

            # inputT[p_i, pi*B + b] = input[b, pi*128 + p_i]
            inputT = const_pool.tile([128, PT * B], F32, tag="inputT")
            for pi in range(PT):
                pt = psum_t_pool.tile([128, 128], F32, tag="tp")
                nc.tensor.transpose(
                    pt[:, :B],
                    input_sb[:, pi * 128 : (pi + 1) * 128],
                    identity[:B, :B],
                )
                nc.vector.tensor_copy(inputT[:, pi * B : (pi + 1) * B], pt[:, :B])

            # wT tiles: wt[pi][p_i, c] = clip(weight[c, pi*128 + p_i], 0, 5)
            wt = [
                wt_pool.tile([128, C], F32, tag=f"wt{pi}", name=f"wt{pi}")
                for pi in range(PT)
            ]
            for ci in range((C + 127) // 128):
                cn = min(128, C - ci * 128)
                wstage = wstage_pool.tile([128, P], F32, tag="wstage")
                nc.sync.dma_start(
                    wstage[:cn, :], weight[ci * 128 : ci * 128 + cn, :]
                )
                for pi in range(PT):
                    pt = psum_t_pool.tile([128, 128], F32, tag="tp")
                    nc.tensor.transpose(
                        pt[:, :cn],
                        wstage[:cn, pi * 128 : (pi + 1) * 128],
                        identity[:cn, :cn],
                    )
                    # fused clip(x, 0, 5) on the PSUM->SBUF copy
                    nc.vector.tensor_scalar(
                        wt[pi][:, ci * 128 : ci * 128 + cn],
                        pt[:, :cn],
                        5.0,
                        0.0,
                        op0=mybir.AluOpType.min,
                        op1=mybir.AluOpType.max,
                    )

            # importance: one [128, PT*C] tile per batch row -> one 4 MB DMA
            for b in range(b_rows):
                out_t = out_pool.tile([128, PT * C], F32, tag="out")
                for pi in range(PT):
                    dst = out_t[:, pi * C : (pi + 1) * C]
                    scal = inputT[:, pi * B + b : pi * B + b + 1]
                    if pi % 3 == 2:
                        nc.scalar.mul(dst, wt[pi][:], scal)
                    else:
                        nc.vector.tensor_scalar_mul(dst, wt[pi][:], scal)
                nc.sync.dma_start(
                    imp[b].rearrange("(pi p) c -> p pi c", p=128),
                    out_t[:].rearrange("p (pi c) -> p pi c", c=C),
                )

            # logits = inputT.T @ wT + bias, accumulated in PSUM
            logits_sb = const_pool.tile([B, C], F32, tag="logits_sb")
            for c0 in range(0, C, 512):
                cw = min(512, C - c0)
                pl = psum_l_pool.tile([B, 512], F32, tag="pl")
                for pi in range(PT):
                    nc.tensor.matmul(
                        pl[:, :cw],
                        lhsT=inputT[:, pi * B : (pi + 1) * B],
                        rhs=wt[pi][:, c0 : c0 + cw],
                        start=(pi == 0),
                        stop=False,
                    )
                nc.tensor.matmul(
                    pl[:, :cw],
                    lhsT=ones[:, :],
                    rhs=bias_sb[:, c0 : c0 + cw],
                    start=False,
                    stop=True,
                )
                nc.scalar.copy(logits_sb[:, c0 : c0 + cw], pl[:, :cw])
            nc.sync.dma_start(logits_d[:], logits_sb[:])

    nc.compile()
    return nc


_PROGRAM = None


def kernel(input, weight, bias):
    global _PROGRAM, LAST_RESULTS

    input = np.ascontiguousarray(input, dtype=np.float32)
    weight = np.ascontiguousarray(weight, dtype=np.float32)
    bias = np.ascontiguousarray(bias, dtype=np.float32)
    assert input.shape == (B_FULL, P) and weight.shape == (C, P)

    if _PROGRAM is None:
        _PROGRAM = _build_program()

    in_maps = [
        {
            "input": input[i * B : (i + 1) * B],
            "weight": weight,
            "bias": bias,
        }
        for i in range(N_CORES)
    ]
    res = run_bass_kernel_spmd(
        _PROGRAM,
        in_maps,
        core_ids=list(range(N_CORES)),
        trace=bool(int(os.environ.get("KERNEL_TRACE", "0"))),
    )
    LAST_RESULTS = res

    importance = np.concatenate([r["importance"] for r in res.results], axis=0)
    logits = np.concatenate([r["logits"] for r in res.results], axis=0)
    return importance, logits
